# revision 1
# baseline (speedup 1.0000x reference)
"""DANet (dual attention) Trainium2 kernel.

Problem shapes (hardcoded): x [4, 64, 64, 64] f32, O = 16, N = H*W = 4096.
Sharding: 8 cores = 4 batches x 2 query-row halves (2048 query rows each).
Each core computes its batch's channel attention (small, duplicated) and
its half of the spatial attention, producing out[b, :, half] directly.

Math notes:
 - spatial softmax: scores lie in [-35, 35] so exp() without max-subtraction
   is safe in f32; the softmax denominator comes free from a fused ones
   column in the att@V matmul (output row 64).
 - channel softmax: softmax(max-E) == exp(rowmin(E)-E)/sum (the reference's
   jax softmax subtracts its own row max), computed in exact fp32.
 - gamma_sa folded into wv/bv host-side; gamma_ca baked as an immediate;
   the "+2x" residual is folded into the out_c matmul as gamma_ca*att_c^T+2I
   (exact fp32 matmul).
 - q/k/v + the two big matmuls run in bf16 (f32 PSUM accumulate); their
   output contribution is scaled by gamma (~0.06-0.1) so the rounding is
   ~1e-3 of the final output. Everything touching the dominant 2x residual
   and the channel-softmax logits stays exact fp32.
"""

import sys

for _p in ("/opt/trn_rl_repo",):
    if _p not in sys.path:
        sys.path.append(_p)

import numpy as np
import ml_dtypes
from contextlib import ExitStack

import concourse.bass as bass
import concourse.bacc as bacc
import concourse.mybir as mybir
import concourse.tile as tile
from concourse import library_config

F32 = mybir.dt.float32
BF16 = mybir.dt.bfloat16
AF = mybir.ActivationFunctionType
AX = mybir.AxisListType
ALU = mybir.AluOpType

B, C, H, W = 4, 64, 64, 64
N = H * W          # 4096
O = C // 4         # 16
NQ = N // 2        # 2048 query rows per core
NK_CH = N // 128   # 32 key chunks of 128


def build_program(gamma_ca: float, repeat: int = 1, loop_n: int = 0):
    nc = bacc.Bacc(
        "TRN2", target_bir_lowering=False, debug=False, num_devices=8
    )

    xf1h_d = nc.dram_tensor("xf1h", [C + 1, N], BF16, kind="ExternalInput").ap()
    xq1h_d = nc.dram_tensor("xq1h", [C + 1, NQ], BF16, kind="ExternalInput").ap()
    xq1_d = nc.dram_tensor("xq1", [C + 1, NQ], F32, kind="ExternalInput").ap()
    xt_d = nc.dram_tensor("xt", [N, C], F32, kind="ExternalInput").ap()
    qT1_d = nc.dram_tensor("qT1", [C + 1, O], BF16, kind="ExternalInput").ap()
    kT1_d = nc.dram_tensor("kT1", [C + 1, O], BF16, kind="ExternalInput").ap()
    wvT1_d = nc.dram_tensor("wvT1", [C + 1, C], BF16, kind="ExternalInput").ap()
    eye_d = nc.dram_tensor("eye64", [C, C], F32, kind="ExternalInput").ap()
    iden2_d = nc.dram_tensor("iden2", [C, C], F32, kind="ExternalInput").ap()
    out_d = nc.dram_tensor("out", [C, NQ], F32, kind="ExternalOutput").ap()

    with tile.TileContext(nc) as tc:
        with ExitStack() as ctx:
            _kernel(
                ctx, tc, gamma_ca,
                xf1h_d, xq1h_d, xq1_d, xt_d, qT1_d, kT1_d, wvT1_d,
                eye_d, iden2_d, out_d, repeat=repeat, loop_n=loop_n,
            )
    nc.compile()
    return nc


def _kernel(ctx, tc, gamma_ca, xf1h_d, xq1h_d, xq1_d, xt_d, qT1_d, kT1_d,
            wvT1_d, eye_d, iden2_d, out_d, repeat=1, loop_n=0):
    nc = tc.nc

    consts = ctx.enter_context(tc.tile_pool(name="consts", bufs=1))
    expp = ctx.enter_context(tc.tile_pool(name="expp", bufs=3))
    sm = ctx.enter_context(tc.tile_pool(name="sm", bufs=2))
    outp = ctx.enter_context(tc.tile_pool(name="outp", bufs=3))
    # PSUM budget: pst 2x2 banks + pacc 2x1 + psmall 2x1 = 8 banks exactly.
    pst = ctx.enter_context(tc.tile_pool(name="pst", bufs=2, space="PSUM"))
    pacc = ctx.enter_context(tc.tile_pool(name="pacc", bufs=2, space="PSUM"))
    psmall = ctx.enter_context(tc.tile_pool(name="psmall", bufs=2, space="PSUM"))

    # Tiny warmup exp so the ACT exp table loads during the DMA preamble.
    warm = sm.tile([1, 16], F32, tag="warm")
    nc.vector.memset(warm[:], 0.0)
    nc.scalar.activation(warm[:], warm[:], AF.Exp)

    def emit_all():
        for _rep in range(repeat):
            _emit_body(
                tc, gamma_ca, consts, expp, sm, outp, pst, pacc, psmall,
                xf1h_d, xq1h_d, xq1_d, xt_d, qT1_d, kT1_d, wvT1_d,
                eye_d, iden2_d, out_d,
            )

    if loop_n:
        with tc.For_i(
            0, loop_n, 1,
            hint_engines=(mybir.EngineType.PE, mybir.EngineType.Activation),
        ):
            emit_all()
    else:
        emit_all()


def _emit_body(tc, gamma_ca, consts, expp, sm, outp, pst, pacc, psmall,
               xf1h_d, xq1h_d, xq1_d, xt_d, qT1_d, kT1_d, wvT1_d,
               eye_d, iden2_d, out_d):
    nc = tc.nc

    # ---- const / input DMAs ----
    # Emission order drives the 8-queue HWDGE round-robin: critical loads
    # (weights, then xf1h/xq1h feeding the q/k/v projections) go first and
    # chunked to engage parallel queues; late-need tensors (xt, xq1 f32,
    # eye/iden2 - all first used >20us in) come after.
    qT1 = consts.tile([C + 1, O], BF16, tag="qT1")
    nc.sync.dma_start(qT1[:], qT1_d[:])
    kT1 = consts.tile([C + 1, O], BF16, tag="kT1")
    nc.sync.dma_start(kT1[:], kT1_d[:])
    wvT1 = consts.tile([C + 1, C], BF16, tag="wvT1")
    nc.sync.dma_start(wvT1[:], wvT1_d[:])

    xf1h = consts.tile([C + 1, N], BF16, tag="xf1h")
    for j in range(4):
        nc.sync.dma_start(
            xf1h[:, j * 1024:(j + 1) * 1024], xf1h_d[:, j * 1024:(j + 1) * 1024]
        )
    xq1h = consts.tile([C + 1, NQ], BF16, tag="xq1h")
    for j in range(2):
        nc.sync.dma_start(
            xq1h[:, j * 1024:(j + 1) * 1024], xq1h_d[:, j * 1024:(j + 1) * 1024]
        )
    # xt sbuf layout: [128, 32*64]; chunk i at cols i*64:(i+1)*64 holds
    # xT rows i*128:(i+1)*128.
    xt = consts.tile([128, NK_CH * C], F32, tag="xt")
    for g in range(4):
        src = xt_d[g * 1024:(g + 1) * 1024, :].rearrange(
            "(c p) f -> p c f", p=128
        )
        dst = xt[:, g * 512:(g + 1) * 512].rearrange("p (c f) -> p c f", f=C)
        nc.sync.dma_start(dst, src)
    xq1 = consts.tile([C + 1, NQ], F32, tag="xq1")
    for j in range(2):
        nc.sync.dma_start(
            xq1[:, j * 1024:(j + 1) * 1024], xq1_d[:, j * 1024:(j + 1) * 1024]
        )
    eye = consts.tile([C, C], F32, tag="eye")
    nc.sync.dma_start(eye[:], eye_d[:])
    iden2 = consts.tile([C, C], F32, tag="iden2")
    nc.sync.dma_start(iden2[:], iden2_d[:])

    # ones row for the denominator broadcast matmul (row 64 used as lhsT,
    # partition-aligned with the accumulator's denominator row).
    ones = consts.tile([C + 1, C], F32, tag="ones")
    nc.vector.memset(ones[:], 1.0)

    # ---- q/k projections (bf16 in, f32 psum, bf16 out) ----
    q_sb = consts.tile([O, NQ], BF16, tag="q")
    for j in range(NQ // 512):
        pq = psmall.tile([O, 512], F32, tag="pp", name=f"pq{j}")
        nc.tensor.matmul(
            pq[:], qT1[:], xq1h[:, j * 512:(j + 1) * 512],
            start=True, stop=True,
        )
        nc.vector.tensor_copy(q_sb[:, j * 512:(j + 1) * 512], pq[:])
    k_sb = consts.tile([O, N], BF16, tag="k")
    for j in range(N // 512):
        pk = psmall.tile([O, 512], F32, tag="pp", name=f"pk{j}")
        nc.tensor.matmul(
            pk[:], kT1[:], xf1h[:, j * 512:(j + 1) * 512],
            start=True, stop=True,
        )
        nc.vector.tensor_copy(k_sb[:, j * 512:(j + 1) * 512], pk[:])

    # ---- vT chunks (bf16), interleaved with a ones column ----
    # vt layout: [128, 32*65]; chunk i = cols i*65..i*65+64 (64 v cols + 1.0).
    vt = consts.tile([128, NK_CH * (C + 1)], BF16, tag="vt")
    vt3 = vt[:].rearrange("p (c u) -> p c u", u=C + 1)
    nc.vector.memset(vt3[:, :, C:C + 1], 1.0)
    for g in range(8):  # 4 chunks per psum tile
        pv = psmall.tile([128, 4 * C], F32, tag="pp", name=f"pv{g}")
        for q4 in range(4):
            i = g * 4 + q4
            nc.tensor.matmul(
                pv[:, q4 * C:(q4 + 1) * C],
                xf1h[:, i * 128:(i + 1) * 128],
                wvT1[:],
                start=True, stop=True,
            )
        dst = vt3[:, g * 4:(g + 1) * 4, 0:C]
        src = pv[:].rearrange("p (c f) -> p c f", f=C)
        nc.vector.tensor_copy(dst, src)

    # ---- main spatial-attention loop: flattened 2-pass stream ----
    attF = sm.tile([C, C], F32, tag="attF")  # written during pass 0
    acc = {}
    ex_tiles = {}

    def emit_st(p, i):
        st = pst.tile([128, 1024], F32, tag="st", name=f"st{p}_{i}")
        for j2 in range(2):
            nc.tensor.matmul(
                st[:, j2 * 512:(j2 + 1) * 512],
                k_sb[:, i * 128:(i + 1) * 128],
                q_sb[:, p * 1024 + j2 * 512:p * 1024 + (j2 + 1) * 512],
                start=True, stop=True,
            )
        ex = expp.tile([128, 1024], BF16, tag="ex", name=f"ex{p}_{i}")
        nc.scalar.activation(ex[:], st[:], AF.Exp)
        ex_tiles[(p, i)] = ex

    def emit_attv(p, i):
        ex = ex_tiles.pop((p, i))
        if i == 0:
            acc[p] = [
                pacc.tile([C + 1, 512], F32, tag="acc", name=f"acc_p{p}_{j}")
                for j in range(2)
            ]
        for j2 in range(2):
            nc.tensor.matmul(
                acc[p][j2][:],
                vt[:, i * (C + 1):(i + 1) * (C + 1)],
                ex[:, j2 * 512:(j2 + 1) * 512],
                start=(i == 0), stop=(i == NK_CH - 1),
            )

    def emit_combine(p):
        for j2 in range(2):
            m0 = p * 1024 + j2 * 512
            a = acc[p][j2]
            rec = sm.tile([C + 1, 512], F32, tag="rec", name=f"rec{p}_{j2}")
            nc.vector.reciprocal(rec[C:C + 1, :], a[C:C + 1, :])
            rbp = psmall.tile([C, 512], F32, tag="pp", name=f"rbp{p}_{j2}")
            nc.tensor.matmul(
                rbp[:], ones[C:C + 1, :], rec[C:C + 1, :],
                start=True, stop=True,
            )
            rb = sm.tile([C, 512], F32, tag="rb", name=f"rb{p}_{j2}")
            nc.vector.tensor_copy(rb[:], rbp[:])
            oc = psmall.tile([C, 512], F32, tag="pp", name=f"oc{p}_{j2}")
            nc.tensor.matmul(
                oc[:], attF[:], xq1[0:C, m0:m0 + 512], start=True, stop=True
            )
            t1 = sm.tile([C, 512], F32, tag="t1", name=f"t1{p}_{j2}")
            nc.vector.tensor_mul(t1[:], a[0:C, :], rb[:])
            ob = outp.tile([C, 512], F32, tag="ob", name=f"ob{p}_{j2}")
            nc.vector.tensor_add(ob[:], t1[:], oc[:])
            nc.sync.dma_start(out_d[:, m0:m0 + 512], ob[:])

    # skew-2 software pipeline across the flattened (pass, chunk) sequence:
    # the PE never head-of-line blocks the next ST behind an exp wait, and
    # the ST/exp stream continues across the pass boundary before combines.
    seq = [(p, i) for p in range(2) for i in range(NK_CH)]
    emit_st(*seq[0])
    emit_st(*seq[1])
    for idx, (p, i) in enumerate(seq):
        if idx + 2 < len(seq):
            emit_st(*seq[idx + 2])
        emit_attv(p, i)
        if p == 0:
            # channel-attention energy matmuls ride the PE slack
            en = _energy_step(nc, psmall, xt, i)
            if i == NK_CH - 1:
                _channel_softmax(
                    nc, tc, sm, psmall, en, eye, iden2, attF, gamma_ca
                )
        if i == NK_CH - 1:
            emit_combine(p)


_EN_TILE = []


def _energy_step(nc, psmall, xt, i):
    if i == 0:
        _EN_TILE.clear()
        _EN_TILE.append(
            psmall.tile([C, C], mybir.dt.float32, tag="pp", name="en")
        )
    en = _EN_TILE[0]
    nc.tensor.matmul(
        en[:], xt[:, i * C:(i + 1) * C], xt[:, i * C:(i + 1) * C],
        start=(i == 0), stop=(i == NK_CH - 1),
    )
    return en


def _channel_softmax(nc, tc, sm, psmall, en, eye, iden2, attF, gamma_ca):
    emin = sm.tile([C, 1], F32, tag="emin")
    nc.vector.tensor_reduce(emin[:], en[:], axis=AX.X, op=ALU.min)
    ae = sm.tile([C, C], F32, tag="ae")
    esum = sm.tile([C, 1], F32, tag="esum")
    nc.scalar.activation(
        ae[:], en[:], AF.Exp, bias=emin[:], scale=-1.0, accum_out=esum[:]
    )
    esr = sm.tile([C, 1], F32, tag="esr")
    nc.vector.reciprocal(esr[:], esum[:])
    ac = sm.tile([C, C], F32, tag="ac")
    nc.vector.tensor_scalar_mul(ac[:], ae[:], esr[:])
    at = psmall.tile([C, C], F32, tag="pp", name="at")
    nc.tensor.transpose(at[:], ac[:], eye[:])
    ats = sm.tile([C, C], F32, tag="ats")
    nc.scalar.mul(ats[:], at[:], float(gamma_ca))
    nc.vector.tensor_add(attF[:], ats[:], iden2[:])


# ---------------- host side ----------------

_PROGRAM_CACHE = {}


def _get_program(gamma_ca: float):
    key = float(gamma_ca)
    if key not in _PROGRAM_CACHE:
        _PROGRAM_CACHE[key] = build_program(key)
    return _PROGRAM_CACHE[key]


def build_in_maps(x, wq, bq, wk, bk, wv, bv, gamma_ca, gamma_sa):
    bf16 = np.dtype(ml_dtypes.bfloat16)
    x = np.asarray(x, dtype=np.float32)
    wq = np.asarray(wq, dtype=np.float32)
    bq = np.asarray(bq, dtype=np.float32)
    wk = np.asarray(wk, dtype=np.float32)
    bk = np.asarray(bk, dtype=np.float32)
    wv = np.asarray(wv, dtype=np.float32)
    bv = np.asarray(bv, dtype=np.float32)
    g_sa = float(np.asarray(gamma_sa).reshape(-1)[0])

    xf = x.reshape(B, C, N)
    ones_row = np.ones((1, N), np.float32)
    qT1 = np.ascontiguousarray(
        np.concatenate([wq.T, bq[None, :]], axis=0).astype(bf16)
    )
    kT1 = np.ascontiguousarray(
        np.concatenate([wk.T, bk[None, :]], axis=0).astype(bf16)
    )
    wvT1 = np.ascontiguousarray(
        (g_sa * np.concatenate([wv.T, bv[None, :]], axis=0)).astype(bf16)
    )
    eye64 = np.eye(C, dtype=np.float32)
    iden2 = 2.0 * np.eye(C, dtype=np.float32)

    in_maps = []
    for core in range(8):
        b, h = core // 2, core % 2
        xf1 = np.concatenate([xf[b], ones_row], axis=0)
        xq1 = np.ascontiguousarray(xf1[:, h * NQ:(h + 1) * NQ])
        in_maps.append({
            "xf1h": np.ascontiguousarray(xf1.astype(bf16)),
            "xq1h": np.ascontiguousarray(xq1.astype(bf16)),
            "xq1": xq1,
            "xt": np.ascontiguousarray(xf[b].T),
            "qT1": qT1,
            "kT1": kT1,
            "wvT1": wvT1,
            "eye64": eye64,
            "iden2": iden2,
        })
    return in_maps


LAST_RESULTS = None


def kernel(x, wq, bq, wk, bk, wv, bv, gamma_ca, gamma_sa):
    global LAST_RESULTS
    from concourse.bass_utils import run_bass_kernel_spmd

    g_ca = float(np.asarray(gamma_ca).reshape(-1)[0])
    nc = _get_program(g_ca)
    in_maps = build_in_maps(x, wq, bq, wk, bk, wv, bv, gamma_ca, gamma_sa)

    res = run_bass_kernel_spmd(nc, in_maps, list(range(8)))
    LAST_RESULTS = res
    out = np.empty((B, C, N), np.float32)
    for core in range(8):
        b, h = core // 2, core % 2
        out[b, :, h * NQ:(h + 1) * NQ] = res.results[core]["out"]
    return out.reshape(B, C, H, W)

